# revision 10
# baseline (speedup 1.0000x reference)
"""Multi-head self-attention (RoPE, causal) on 8 Trainium2 NeuronCores.

v2 — HAM-warm interleaved schedule.

Sharding: tensor-parallel over heads (2 of 16 per core), token-sharded
output projection after an AllToAll.

Key structure vs v1:
  - everything bf16 on the SBUF side (q/k/v, cos/sin, probabilities),
    fp32 only inside PSUM accumulators
  - attention processes tq in 512-wide chunks (cq); per tk-block j one
    scores matmul per head into a paired psum tile [128, 2head, 512] so
    softmax exp of BOTH heads is a single ACT instruction
  - projection / output-projection matmuls are interleaved into the
    attention j-loop as PE filler so the tensor engine never idles and
    the HAM clock gate stays at 8/8 (2.4 GHz)
  - softmax normalize on GPSIMD (partition_broadcast + tensor divide),
    keeping ACT free for exp and DVE for RoPE
  - AllToAll per half-batch (8 collectives) so output projection
    overlaps attention and the tail is one small collective
"""

import numpy as np
import ml_dtypes

import concourse.bacc as bacc
import concourse.mybir as mybir
import concourse.tile as tile
from concourse import bass_utils
from concourse.masks import make_identity

F32 = mybir.dt.float32
BF16 = mybir.dt.bfloat16
MULT = mybir.AluOpType.mult
ADD = mybir.AluOpType.add

B, T, D = 4, 2048, 1024
H, DH = 16, 64
N_CORES = 8
HPC = H // N_CORES            # heads per core = 2
EC = HPC * DH                 # feature slice per core = 128
NT = B * T                    # 8192 tokens
NB = T // 128                 # 16 tk blocks per batch
THETA = 10000.0

_CACHE = {}
last_results = None


def _build_program():
    nc = bacc.Bacc("TRN2", debug=False, target_bir_lowering=False,
                   num_devices=N_CORES)

    xt_d = nc.dram_tensor("xt", [128, 8, NT], BF16, kind="ExternalInput")
    wq_d = nc.dram_tensor("wq", [128, 8, EC], BF16, kind="ExternalInput")
    wk_d = nc.dram_tensor("wk", [128, 8, EC], BF16, kind="ExternalInput")
    wv_d = nc.dram_tensor("wv", [128, 8, EC], BF16, kind="ExternalInput")
    wo_d = nc.dram_tensor("wo", [128, 8, D], BF16, kind="ExternalInput")
    cos_d = nc.dram_tensor("cosb", [128, T], BF16, kind="ExternalInput")
    sin_d = nc.dram_tensor("sinb", [128, T], BF16, kind="ExternalInput")
    rotm_d = nc.dram_tensor("rotm", [128, 128], BF16, kind="ExternalInput")
    tri_d = nc.dram_tensor("trimask", [128, 128], BF16, kind="ExternalInput")
    y_d = nc.dram_tensor("y", [B * 256, D], F32, kind="ExternalOutput")

    with tile.TileContext(nc) as tc:
        with (
            tc.tile_pool(name="consts", bufs=1) as consts,
            tc.tile_pool(name="big", bufs=1) as big,
            tc.tile_pool(name="xp", bufs=2) as xp,
            tc.tile_pool(name="stq", bufs=3) as stq,
            tc.tile_pool(name="stt", bufs=3) as stt,
            tc.tile_pool(name="expp", bufs=4) as expp,
            tc.tile_pool(name="nrm", bufs=2) as nrm,
            tc.tile_pool(name="outp", bufs=2) as outp,
            tc.tile_pool(name="wop", bufs=2) as wop,
            tc.tile_pool(name="sppA", bufs=1, space="PSUM") as sppA,
            tc.tile_pool(name="sppB", bufs=1, space="PSUM") as sppB,
            tc.tile_pool(name="pvp", bufs=2, space="PSUM") as pvp,
            tc.tile_pool(name="pp", bufs=2, space="PSUM") as pp,
            tc.tile_pool(name="dram", bufs=16, space="DRAM") as dram,
        ):
            # ---- constants ----
            cos_sb = consts.tile([128, T], BF16)
            sin_sb = consts.tile([128, T], BF16)
            rotm_sb = consts.tile([128, 128], BF16)
            tri_sb = consts.tile([128, 128], BF16)
            ident_sb = consts.tile([128, 128], BF16)
            wq_sb = consts.tile([128, 8, EC], BF16)
            wk_sb = consts.tile([128, 8, EC], BF16)
            wv_sb = consts.tile([128, 8, EC], BF16)
            wo_sb = consts.tile([128, 8, D], BF16)
            nc.sync.dma_start(wq_sb[:], wq_d[:, :, :])
            nc.sync.dma_start(cos_sb[:], cos_d[:, :])
            nc.sync.dma_start(sin_sb[:], sin_d[:, :])
            nc.sync.dma_start(rotm_sb[:], rotm_d[:, :])
            nc.sync.dma_start(tri_sb[:], tri_d[:, :])
            make_identity(nc, ident_sb[:])
            nc.sync.dma_start(wk_sb[:], wk_d[:, :, :])
            nc.sync.dma_start(wv_sb[:], wv_d[:, :, :])
            nc.sync.dma_start(wo_sb[:], wo_d[:, :, :])

            # ---- persistent tensors ----
            qT = big.tile([128, NT], BF16, tag="qT")
            kT = big.tile([128, NT], BF16, tag="kT")
            # V per (pair, tk-block): [tk=128, 65] with ones in col 64
            vext = big.tile([128, HPC * B, NB, 65], BF16, tag="vext")
            nc.vector.memset(vext[:, :, :, 64], 1.0)

            # AllToAll buffers: one per (batch, half); [dest, feat, tok]
            a2a_in = [dram.tile([N_CORES, 128, 128], BF16, tag="a2a",
                                name=f"a2ai{i}") for i in range(8)]
            a2a_out = [dram.tile([N_CORES, 128, 128], BF16, tag="a2a",
                                 name=f"a2ao{i}") for i in range(8)]
            rg = [list(range(N_CORES))]

            # ============ phase-1 chunk (512 tokens): QKV + RoPE ==========
            def chunk_gen(ci):
                t0 = 512 * ci
                bb = t0 // T
                s0 = t0 % T
                xt = xp.tile([128, 8, 512], BF16, tag="x")
                nc.sync.dma_start(xt[:], xt_d[:, :, t0:t0 + 512])
                yield

                def proj_mm(w_sb, prw, k0):
                    for ko in range(k0, k0 + 4):
                        nc.tensor.matmul(prw, w_sb[:, ko, :], xt[:, ko, :],
                                         start=(ko == 0), stop=(ko == 7))

                def rope_tail(prw, dest):
                    raw = stq.tile([128, 512], BF16, tag="raw", name="raw")
                    nc.scalar.copy(raw[:], prw)
                    rot = pp.tile([128, 512], F32, tag="pp", name="rot")
                    nc.tensor.matmul(rot, rotm_sb[:], raw[:],
                                     start=True, stop=True)
                    t1 = stt.tile([128, 512], BF16, tag="t1")
                    nc.vector.tensor_tensor(
                        t1[:], raw[:], cos_sb[:, s0:s0 + 512], MULT)
                    t2 = stt.tile([128, 512], BF16, tag="t2")
                    nc.vector.tensor_tensor(
                        t2[:], rot[:], sin_sb[:, s0:s0 + 512], MULT)
                    nc.vector.tensor_tensor(
                        dest[:, t0:t0 + 512], t1[:], t2[:], ADD)

                pq = pp.tile([128, 512], F32, tag="pp", name="pq")
                proj_mm(wq_sb, pq, 0)
                yield
                proj_mm(wq_sb, pq, 4)
                yield
                rope_tail(pq, qT)
                yield
                pk = pp.tile([128, 512], F32, tag="pp", name="pk")
                proj_mm(wk_sb, pk, 0)
                yield
                proj_mm(wk_sb, pk, 4)
                yield
                rope_tail(pk, kT)
                yield
                pv_ = pp.tile([128, 512], F32, tag="pp", name="pvv")
                proj_mm(wv_sb, pv_, 0)
                yield
                proj_mm(wv_sb, pv_, 4)
                vraw = stq.tile([128, 512], BF16, tag="vraw", name="vraw")
                nc.vector.tensor_copy(vraw[:], pv_)
                yield
                for h in range(HPC):
                    pair = bb * HPC + h
                    for bi in range(4):
                        jg = s0 // 128 + bi
                        tp = pp.tile([128, 1024], BF16, tag="pp",
                                     name="vtr")[:, 0:64]
                        nc.tensor.transpose(
                            tp,
                            vraw[64 * h:64 * h + 64,
                                 128 * bi:128 * bi + 128],
                            ident_sb[64 * h:64 * h + 64,
                                     64 * h:64 * h + 64])
                        nc.vector.tensor_copy(vext[:, pair, jg, 0:64], tp)
                    yield

            # ============ output projection for one (batch, half) =========
            def oproj_gen(bb, hf):
                oall = wop.tile([128, 8, 128], BF16, tag="oall")
                nc.sync.dma_start(
                    oall[:], a2a_out[2 * bb + hf][:].rearrange(
                        "s p t -> p s t"))
                yield
                for eo in range(2):
                    ot = pp.tile([128, 512], F32, tag="pp", name="ot")
                    for ec in range(8):
                        nc.tensor.matmul(
                            ot, oall[:, ec, :],
                            wo_sb[:, ec, 512 * eo:512 * eo + 512],
                            start=(ec == 0), stop=(ec == 7))
                    ys = outp.tile([128, 512], F32, tag="ys")
                    nc.scalar.copy(ys[:], ot)
                    nc.sync.dma_start(
                        y_d[256 * bb + 128 * hf:256 * bb + 128 * hf + 128,
                            512 * eo:512 * eo + 512], ys[:])
                    yield

            def delay_gen(n):
                for _ in range(n):
                    yield

            # ============ attention for one batch =========================
            def take(fillers, n):
                done = 0
                while done < n and fillers:
                    try:
                        next(fillers[0])
                        done += 1
                    except StopIteration:
                        fillers.pop(0)

            def do_attn(bb, fillers, cq_order=(0, 1, 2, 3)):
                tb0 = bb * T
                done = set()
                for cq in cq_order:
                    q0 = tb0 + 512 * cq
                    jmax = 4 * (cq + 1)
                    pvt = [pvp.tile([65, 512], F32, tag="pv",
                                    name=f"pv{hh}") for hh in range(2)]

                    def scores(j, pool):
                        spt = pool.tile([128, 2, 512], F32, tag="s",
                                        name="spt")
                        lo = max(0, 128 * j - 512 * cq)
                        for hh in range(2):
                            nc.tensor.matmul(
                                spt[:, hh, lo:512],
                                kT[64 * hh:64 * hh + 64,
                                   tb0 + 128 * j:tb0 + 128 * j + 128],
                                qT[64 * hh:64 * hh + 64, q0 + lo:q0 + 512],
                                start=True, stop=True)
                        return spt, lo

                    def pv_mm(pj, pex, plo, last):
                        for hh in range(2):
                            nc.tensor.matmul(
                                pvt[hh][:, plo:512],
                                vext[:, bb * HPC + hh, pj, :],
                                pex[:, hh, plo:512],
                                start=(pj == 0), stop=last)

                    spt, lo = scores(0, sppA)
                    prev = None
                    for j in range(jmax):
                        ex = expp.tile([128, 2, 512], BF16, tag="e",
                                       name="ex")
                        nc.scalar.activation(
                            ex[:, :, lo:512], spt[:, :, lo:512],
                            mybir.ActivationFunctionType.Exp, scale=0.125)
                        if j >= 4 * cq:   # diagonal block: causal mask
                            for hh in range(2):
                                nc.vector.tensor_tensor(
                                    ex[:, hh, lo:lo + 128],
                                    ex[:, hh, lo:lo + 128],
                                    tri_sb[:], MULT)
                        if prev is not None:
                            pv_mm(*prev, last=False)
                        take(fillers, 1)
                        prev = (j, ex, lo)
                        if j + 1 < jmax:
                            spt, lo = scores(j + 1,
                                             sppB if (j + 1) % 2 else sppA)
                    pv_mm(*prev, last=True)

                    # normalize: pack den to [128,8] via DRAM bounce, DVE
                    # reciprocal at free-size 8, unpack + broadcast via DMA,
                    # multiply on DVE; ship to a2a_in
                    den = nrm.tile([1, 1024], F32, tag="den")
                    unn = []
                    for hh in range(2):
                        u = nrm.tile([64, 512], BF16, tag=f"unn{hh}",
                                     name="unn")
                        nc.scalar.copy(u[:], pvt[hh][0:64, :])
                        nc.vector.tensor_copy(den[0:1, 512 * hh:512 * hh + 512],
                                              pvt[hh][64:65, :])
                        unn.append(u)
                    dscr = dram.tile([1, 1024], F32, tag="dscr", name="dscr")
                    nc.sync.dma_start(dscr[:], den[:])
                    dent = nrm.tile([128, 8], F32, tag="dent")
                    nc.sync.dma_start(
                        dent[:, :],
                        dscr[0:1, :].rearrange("o (c p) -> (o p) c", p=128))
                    rect = nrm.tile([128, 8], BF16, tag="rect")
                    with nc.allow_low_precision(reason="softmax denom bf16"):
                        nc.vector.reciprocal(rect[:], dent[:])
                    drec = dram.tile([1, 1024], BF16, tag="drec", name="drec")
                    nc.sync.dma_start(
                        drec[0:1, :].rearrange("o (c p) -> (o p) c", p=128),
                        rect[:, :])
                    dbb = nrm.tile([64, 1024], BF16, tag="dbb")
                    nc.sync.dma_start(dbb[:],
                                      drec[0:1, :].to_broadcast((64, 1024)))
                    take(fillers, 2)
                    hf = cq // 2
                    for hh in range(2):
                        ao = nrm.tile([64, 512], BF16, tag=f"ao{hh}",
                                      name="ao")
                        nc.vector.tensor_tensor(
                            ao[:], unn[hh][:],
                            dbb[:, 512 * hh:512 * hh + 512], MULT)
                        u0 = 512 * (cq % 2)
                        for q4 in range(4):
                            dd = (u0 + 128 * q4) // 128
                            nc.sync.dma_start(
                                a2a_in[2 * bb + hf][dd,
                                                    64 * hh:64 * hh + 64, :],
                                ao[:, 128 * q4:128 * q4 + 128])
                    take(fillers, 1)
                    done.add(cq)
                    if 2 * hf in done and 2 * hf + 1 in done:
                        nc.gpsimd.collective_compute(
                            "AllToAll", mybir.AluOpType.bypass,
                            replica_groups=rg,
                            ins=[a2a_in[2 * bb + hf].opt()],
                            outs=[a2a_out[2 * bb + hf].opt()])
                take(fillers, 10000)

            # ============ schedule ========================================
            for ci in range(4):
                for _ in chunk_gen(ci):
                    pass
            for bb in range(4):
                c0 = 4 * bb + 4
                if bb == 0:
                    fill = [chunk_gen(c) for c in range(c0, c0 + 4)]
                elif bb < 3:
                    fill = [chunk_gen(c0), oproj_gen(bb - 1, 0),
                            chunk_gen(c0 + 1), chunk_gen(c0 + 2),
                            oproj_gen(bb - 1, 1), chunk_gen(c0 + 3)]
                else:
                    fill = [delay_gen(11), oproj_gen(2, 0),
                            delay_gen(16), oproj_gen(2, 1)]
                do_attn(bb, fill,
                        cq_order=(2, 3, 0, 1) if bb == 3 else (0, 1, 2, 3))
            for hf in (1, 0):
                for _ in oproj_gen(3, hf):
                    pass

    nc.compile()
    return nc


def _host_inputs(x, Wq, Wk, Wv, Wo, token_positions):
    """Per-core in_maps with transposed/tiled layouts."""
    x = np.asarray(x, dtype=np.float32)
    xt_bf = np.ascontiguousarray(
        x.reshape(NT, D).T.reshape(8, 128, NT).transpose(1, 0, 2)
    ).astype(ml_dtypes.bfloat16)

    pos = np.asarray(token_positions).astype(np.float64)
    inv_freq = 1.0 / (THETA ** (np.arange(0, DH, 2, dtype=np.float64) / DH))
    ang = pos[None, :] * inv_freq[:, None]          # [32, T]
    cos_p = np.cos(ang)
    sin_p = np.sin(ang)
    # partition p (0..127): within-head dim d = p % 64, pair = d // 2
    d_idx = (np.arange(128) % 64) // 2
    cosb = cos_p[d_idx, :].astype(ml_dtypes.bfloat16)
    sinb = sin_p[d_idx, :].astype(ml_dtypes.bfloat16)

    rotm = np.zeros((128, 128), dtype=np.float32)
    for i in range(64):
        rotm[2 * i + 1, 2 * i] = -1.0   # rot[2i] = -in[2i+1]
        rotm[2 * i, 2 * i + 1] = 1.0    # rot[2i+1] = in[2i]
    rotm = rotm.astype(ml_dtypes.bfloat16)
    tri = np.tril(np.ones((128, 128), dtype=np.float32)).T  # [tk, tq] tk<=tq
    tri = tri.astype(ml_dtypes.bfloat16)

    def wtiles(W, sl):
        # lhsT tiles: [p, ko, e] with d = ko*128+p contracting
        Wt = np.ascontiguousarray(W[sl, :].T)        # [D, e]
        return np.ascontiguousarray(
            Wt.reshape(8, 128, Wt.shape[1]).transpose(1, 0, 2))

    WoT = np.ascontiguousarray(np.asarray(Wo, dtype=np.float32).T)
    wo_t = np.ascontiguousarray(WoT.reshape(8, 128, D).transpose(1, 0, 2))

    in_maps = []
    for c in range(N_CORES):
        sl = slice(EC * c, EC * (c + 1))
        in_maps.append({
            "xt": xt_bf,
            "wq": wtiles(np.asarray(Wq, np.float32), sl).astype(ml_dtypes.bfloat16),
            "wk": wtiles(np.asarray(Wk, np.float32), sl).astype(ml_dtypes.bfloat16),
            "wv": wtiles(np.asarray(Wv, np.float32), sl).astype(ml_dtypes.bfloat16),
            "wo": wo_t.astype(ml_dtypes.bfloat16),
            "cosb": cosb,
            "sinb": sinb,
            "rotm": rotm,
            "trimask": tri,
        })
    return in_maps


def kernel(x, Wq, Wk, Wv, Wo, token_positions):
    global last_results
    if "nc" not in _CACHE:
        _CACHE["nc"] = _build_program()
    nc = _CACHE["nc"]
    in_maps = _host_inputs(x, Wq, Wk, Wv, Wo, token_positions)
    res = bass_utils.run_bass_kernel_spmd(nc, in_maps, list(range(N_CORES)))
    last_results = res
    y = np.empty((B, T, D), dtype=np.float32)
    for c in range(N_CORES):
        yc = res.results[c]["y"]          # [B*256, D]
        for bb in range(B):
            for hf in range(2):
                r0 = 256 * bb + 128 * hf
                t0 = 1024 * hf + 128 * c
                y[bb, t0:t0 + 128, :] = yc[r0:r0 + 128, :]
    return y


# revision 12
# speedup vs baseline: 1.8883x; 1.8883x over previous
"""Multi-head self-attention (RoPE, causal) on 8 Trainium2 NeuronCores.

v2 — HAM-warm interleaved schedule.

Sharding: tensor-parallel over heads (2 of 16 per core), token-sharded
output projection after an AllToAll.

Key structure vs v1:
  - everything bf16 on the SBUF side (q/k/v, cos/sin, probabilities),
    fp32 only inside PSUM accumulators
  - attention processes tq in 512-wide chunks (cq); per tk-block j one
    scores matmul per head into a paired psum tile [128, 2head, 512] so
    softmax exp of BOTH heads is a single ACT instruction
  - projection / output-projection matmuls are interleaved into the
    attention j-loop as PE filler so the tensor engine never idles and
    the HAM clock gate stays at 8/8 (2.4 GHz)
  - softmax normalize on GPSIMD (partition_broadcast + tensor divide),
    keeping ACT free for exp and DVE for RoPE
  - AllToAll per half-batch (8 collectives) so output projection
    overlaps attention and the tail is one small collective
"""

import numpy as np
import ml_dtypes

import concourse.bacc as bacc
import concourse.mybir as mybir
import concourse.tile as tile
from concourse import bass_utils
from concourse.masks import make_identity

F32 = mybir.dt.float32
BF16 = mybir.dt.bfloat16
MULT = mybir.AluOpType.mult
ADD = mybir.AluOpType.add

B, T, D = 4, 2048, 1024
H, DH = 16, 64
N_CORES = 8
HPC = H // N_CORES            # heads per core = 2
EC = HPC * DH                 # feature slice per core = 128
NT = B * T                    # 8192 tokens
NB = T // 128                 # 16 tk blocks per batch
THETA = 10000.0

_CACHE = {}
last_results = None


def _build_program():
    nc = bacc.Bacc("TRN2", debug=False, target_bir_lowering=False,
                   num_devices=N_CORES)

    xt_d = nc.dram_tensor("xt", [128, 8, NT], BF16, kind="ExternalInput")
    wq_d = nc.dram_tensor("wq", [128, 8, EC], BF16, kind="ExternalInput")
    wk_d = nc.dram_tensor("wk", [128, 8, EC], BF16, kind="ExternalInput")
    wv_d = nc.dram_tensor("wv", [128, 8, EC], BF16, kind="ExternalInput")
    wo_d = nc.dram_tensor("wo", [128, 8, D], BF16, kind="ExternalInput")
    cos_d = nc.dram_tensor("cosb", [128, T], BF16, kind="ExternalInput")
    sin_d = nc.dram_tensor("sinb", [128, T], BF16, kind="ExternalInput")
    rotm_d = nc.dram_tensor("rotm", [128, 128], BF16, kind="ExternalInput")
    tri_d = nc.dram_tensor("trimask", [128, 128], BF16, kind="ExternalInput")
    y_d = nc.dram_tensor("y", [B * 256, D], F32, kind="ExternalOutput")

    with tile.TileContext(nc) as tc:
        with (
            tc.tile_pool(name="consts", bufs=1) as consts,
            tc.tile_pool(name="big", bufs=1) as big,
            tc.tile_pool(name="xp", bufs=2) as xp,
            tc.tile_pool(name="stq", bufs=3) as stq,
            tc.tile_pool(name="stt", bufs=3) as stt,
            tc.tile_pool(name="expp", bufs=4) as expp,
            tc.tile_pool(name="nrm", bufs=2) as nrm,
            tc.tile_pool(name="outp", bufs=2) as outp,
            tc.tile_pool(name="wop", bufs=2) as wop,
            tc.tile_pool(name="sppA", bufs=1, space="PSUM") as sppA,
            tc.tile_pool(name="sppB", bufs=1, space="PSUM") as sppB,
            tc.tile_pool(name="pvp", bufs=2, space="PSUM") as pvp,
            tc.tile_pool(name="pp", bufs=2, space="PSUM") as pp,
            tc.tile_pool(name="dram", bufs=16, space="DRAM") as dram,
        ):
            # ---- constants ----
            cos_sb = consts.tile([128, T], BF16)
            sin_sb = consts.tile([128, T], BF16)
            rotm_sb = consts.tile([128, 128], BF16)
            tri_sb = consts.tile([128, 128], BF16)
            ident_sb = consts.tile([128, 128], BF16)
            identf_sb = consts.tile([128, 128], F32)
            wq_sb = consts.tile([128, 8, EC], BF16)
            wk_sb = consts.tile([128, 8, EC], BF16)
            wv_sb = consts.tile([128, 8, EC], BF16)
            wo_sb = consts.tile([128, 8, D], BF16)
            nc.sync.dma_start(wq_sb[:], wq_d[:, :, :])
            nc.sync.dma_start(cos_sb[:], cos_d[:, :])
            nc.sync.dma_start(sin_sb[:], sin_d[:, :])
            nc.sync.dma_start(rotm_sb[:], rotm_d[:, :])
            nc.sync.dma_start(tri_sb[:], tri_d[:, :])
            make_identity(nc, ident_sb[:])
            make_identity(nc, identf_sb[:])
            nc.sync.dma_start(wk_sb[:], wk_d[:, :, :])
            nc.sync.dma_start(wv_sb[:], wv_d[:, :, :])
            nc.sync.dma_start(wo_sb[:], wo_d[:, :, :])

            # ---- persistent tensors ----
            qT = big.tile([128, NT], BF16, tag="qT")
            kT = big.tile([128, NT], BF16, tag="kT")
            # V per (pair, tk-block): [tk=128, 65] with ones in col 64
            vext = big.tile([128, HPC * B, NB, 65], BF16, tag="vext")
            nc.vector.memset(vext[:, :, :, 64], 1.0)

            # AllToAll buffers: one per (batch, half); [dest, feat, tok]
            a2a_in = [dram.tile([N_CORES, 128, 128], BF16, tag="a2a",
                                name=f"a2ai{i}") for i in range(8)]
            a2a_out = [dram.tile([N_CORES, 128, 128], BF16, tag="a2a",
                                 name=f"a2ao{i}") for i in range(8)]
            rg = [list(range(N_CORES))]

            # ============ phase-1 chunk (512 tokens): QKV + RoPE ==========
            def chunk_gen(ci):
                t0 = 512 * ci
                bb = t0 // T
                s0 = t0 % T
                xt = xp.tile([128, 8, 512], BF16, tag="x")
                nc.sync.dma_start(xt[:], xt_d[:, :, t0:t0 + 512])
                yield

                def proj_mm(w_sb, prw, k0):
                    for ko in range(k0, k0 + 4):
                        nc.tensor.matmul(prw, w_sb[:, ko, :], xt[:, ko, :],
                                         start=(ko == 0), stop=(ko == 7))

                def rope_tail(prw, dest):
                    raw = stq.tile([128, 512], BF16, tag="raw", name="raw")
                    nc.scalar.copy(raw[:], prw)
                    rot = pp.tile([128, 512], F32, tag="pp", name="rot")
                    nc.tensor.matmul(rot, rotm_sb[:], raw[:],
                                     start=True, stop=True)
                    t1 = stt.tile([128, 512], BF16, tag="t1")
                    nc.vector.tensor_tensor(
                        t1[:], raw[:], cos_sb[:, s0:s0 + 512], MULT)
                    t2 = stt.tile([128, 512], BF16, tag="t2")
                    nc.vector.tensor_tensor(
                        t2[:], rot[:], sin_sb[:, s0:s0 + 512], MULT)
                    nc.vector.tensor_tensor(
                        dest[:, t0:t0 + 512], t1[:], t2[:], ADD)

                pq = pp.tile([128, 512], F32, tag="pp", name="pq")
                proj_mm(wq_sb, pq, 0)
                yield
                proj_mm(wq_sb, pq, 4)
                yield
                rope_tail(pq, qT)
                yield
                pk = pp.tile([128, 512], F32, tag="pp", name="pk")
                proj_mm(wk_sb, pk, 0)
                yield
                proj_mm(wk_sb, pk, 4)
                yield
                rope_tail(pk, kT)
                yield
                pv_ = pp.tile([128, 512], F32, tag="pp", name="pvv")
                proj_mm(wv_sb, pv_, 0)
                yield
                proj_mm(wv_sb, pv_, 4)
                vraw = stq.tile([128, 512], BF16, tag="vraw", name="vraw")
                nc.vector.tensor_copy(vraw[:], pv_)
                yield
                for h in range(HPC):
                    pair = bb * HPC + h
                    for bi in range(4):
                        jg = s0 // 128 + bi
                        tp = pp.tile([128, 1024], BF16, tag="pp",
                                     name="vtr")[:, 0:64]
                        nc.tensor.transpose(
                            tp,
                            vraw[64 * h:64 * h + 64,
                                 128 * bi:128 * bi + 128],
                            ident_sb[64 * h:64 * h + 64,
                                     64 * h:64 * h + 64])
                        nc.vector.tensor_copy(vext[:, pair, jg, 0:64], tp)
                    yield

            # ============ output projection for one (batch, half) =========
            def oproj_gen(bb, hf):
                oall = wop.tile([128, 8, 128], BF16, tag="oall")
                nc.sync.dma_start(
                    oall[:], a2a_out[2 * bb + hf][:].rearrange(
                        "s p t -> p s t"))
                yield
                for eo in range(2):
                    ot = pp.tile([128, 512], F32, tag="pp", name="ot")
                    for ec in range(8):
                        nc.tensor.matmul(
                            ot, oall[:, ec, :],
                            wo_sb[:, ec, 512 * eo:512 * eo + 512],
                            start=(ec == 0), stop=(ec == 7))
                    ys = outp.tile([128, 512], F32, tag="ys")
                    nc.scalar.copy(ys[:], ot)
                    nc.sync.dma_start(
                        y_d[256 * bb + 128 * hf:256 * bb + 128 * hf + 128,
                            512 * eo:512 * eo + 512], ys[:])
                    yield

            def delay_gen(n):
                for _ in range(n):
                    yield

            # ============ attention for one batch =========================
            def take(fillers, n):
                done = 0
                while done < n and fillers:
                    try:
                        next(fillers[0])
                        done += 1
                    except StopIteration:
                        fillers.pop(0)

            def do_attn(bb, fillers, cq_order=(0, 1, 2, 3)):
                tb0 = bb * T
                done = set()
                for cq in cq_order:
                    q0 = tb0 + 512 * cq
                    jmax = 4 * (cq + 1)
                    pvt = [pvp.tile([65, 512], F32, tag="pv",
                                    name=f"pv{hh}") for hh in range(2)]

                    def scores(j, pool):
                        spt = pool.tile([128, 2, 512], F32, tag="s",
                                        name="spt")
                        lo = max(0, 128 * j - 512 * cq)
                        for hh in range(2):
                            nc.tensor.matmul(
                                spt[:, hh, lo:512],
                                kT[64 * hh:64 * hh + 64,
                                   tb0 + 128 * j:tb0 + 128 * j + 128],
                                qT[64 * hh:64 * hh + 64, q0 + lo:q0 + 512],
                                start=True, stop=True)
                        return spt, lo

                    def pv_mm(pj, pex, plo, last):
                        for hh in range(2):
                            nc.tensor.matmul(
                                pvt[hh][:, plo:512],
                                vext[:, bb * HPC + hh, pj, :],
                                pex[:, hh, plo:512],
                                start=(pj == 0), stop=last)

                    spt, lo = scores(0, sppA)
                    prev = None
                    for j in range(jmax):
                        ex = expp.tile([128, 2, 512], BF16, tag="e",
                                       name="ex")
                        nc.scalar.activation(
                            ex[:, :, lo:512], spt[:, :, lo:512],
                            mybir.ActivationFunctionType.Exp, scale=0.125)
                        if j >= 4 * cq:   # diagonal block: causal mask
                            for hh in range(2):
                                nc.vector.tensor_tensor(
                                    ex[:, hh, lo:lo + 128],
                                    ex[:, hh, lo:lo + 128],
                                    tri_sb[:], MULT)
                        if prev is not None:
                            pv_mm(*prev, last=False)
                        take(fillers, 1)
                        prev = (j, ex, lo)
                        if j + 1 < jmax:
                            spt, lo = scores(j + 1,
                                             sppB if (j + 1) % 2 else sppA)
                    pv_mm(*prev, last=True)

                    # normalize: PE-transpose den to [128,8], DVE reciprocal
                    # at free-size 8, PE-transpose back, DMA broadcast via
                    # DRAM, multiply on DVE; ship to a2a_in
                    den = nrm.tile([1, 1024], F32, tag="den")
                    unn = []
                    for hh in range(2):
                        u = nrm.tile([64, 512], BF16, tag=f"unn{hh}",
                                     name="unn")
                        nc.scalar.copy(u[:], pvt[hh][0:64, :])
                        nc.vector.tensor_copy(den[0:1, 512 * hh:512 * hh + 512],
                                              pvt[hh][64:65, :])
                        unn.append(u)
                    dent = pp.tile([128, 512], F32, tag="pp",
                                   name="dent")[:, 0:8]
                    for c8 in range(8):
                        nc.tensor.transpose(
                            dent[:, c8:c8 + 1],
                            den[0:1, 128 * c8:128 * c8 + 128],
                            identf_sb[0:1, 0:1])
                    rect = nrm.tile([128, 8], F32, tag="rect")
                    nc.vector.reciprocal(rect[:], dent[:])
                    rec8 = pp.tile([128, 512], F32, tag="pp",
                                   name="rec8")[0:8, 0:128]
                    nc.tensor.transpose(rec8, rect[:, :], identf_sb[:, :])
                    rec8_sb = nrm.tile([8, 128], BF16, tag="rec8sb")
                    nc.vector.tensor_copy(rec8_sb[:], rec8)
                    drec = dram.tile([1, 1024], BF16, tag="drec", name="drec")
                    nc.sync.dma_start(
                        drec[0:1, :].rearrange("o (c p) -> (o c) p", p=128),
                        rec8_sb[:, :])
                    dbb = nrm.tile([64, 1024], BF16, tag="dbb")
                    nc.sync.dma_start(dbb[:],
                                      drec[0:1, :].to_broadcast((64, 1024)))
                    take(fillers, 2)
                    hf = cq // 2
                    for hh in range(2):
                        ao = nrm.tile([64, 512], BF16, tag=f"ao{hh}",
                                      name="ao")
                        nc.vector.tensor_tensor(
                            ao[:], unn[hh][:],
                            dbb[:, 512 * hh:512 * hh + 512], MULT)
                        u0 = 512 * (cq % 2)
                        for q4 in range(4):
                            dd = (u0 + 128 * q4) // 128
                            nc.sync.dma_start(
                                a2a_in[2 * bb + hf][dd,
                                                    64 * hh:64 * hh + 64, :],
                                ao[:, 128 * q4:128 * q4 + 128])
                    take(fillers, 1)
                    done.add(cq)
                    if 2 * hf in done and 2 * hf + 1 in done:
                        nc.gpsimd.collective_compute(
                            "AllToAll", mybir.AluOpType.bypass,
                            replica_groups=rg,
                            ins=[a2a_in[2 * bb + hf].opt()],
                            outs=[a2a_out[2 * bb + hf].opt()])
                take(fillers, 10000)

            # ============ schedule ========================================
            for ci in range(4):
                for _ in chunk_gen(ci):
                    pass
            for bb in range(4):
                c0 = 4 * bb + 4
                if bb == 0:
                    fill = [chunk_gen(c) for c in range(c0, c0 + 4)]
                elif bb < 3:
                    fill = [chunk_gen(c0), oproj_gen(bb - 1, 0),
                            chunk_gen(c0 + 1), chunk_gen(c0 + 2),
                            oproj_gen(bb - 1, 1), chunk_gen(c0 + 3)]
                else:
                    fill = [delay_gen(11), oproj_gen(2, 0),
                            delay_gen(16), oproj_gen(2, 1)]
                do_attn(bb, fill,
                        cq_order=(2, 3, 0, 1) if bb == 3 else (0, 1, 2, 3))
            for hf in (1, 0):
                for _ in oproj_gen(3, hf):
                    pass

    nc.compile()
    return nc


def _host_inputs(x, Wq, Wk, Wv, Wo, token_positions):
    """Per-core in_maps with transposed/tiled layouts."""
    x = np.asarray(x, dtype=np.float32)
    xt_bf = np.ascontiguousarray(
        x.reshape(NT, D).T.reshape(8, 128, NT).transpose(1, 0, 2)
    ).astype(ml_dtypes.bfloat16)

    pos = np.asarray(token_positions).astype(np.float64)
    inv_freq = 1.0 / (THETA ** (np.arange(0, DH, 2, dtype=np.float64) / DH))
    ang = pos[None, :] * inv_freq[:, None]          # [32, T]
    cos_p = np.cos(ang)
    sin_p = np.sin(ang)
    # partition p (0..127): within-head dim d = p % 64, pair = d // 2
    d_idx = (np.arange(128) % 64) // 2
    cosb = cos_p[d_idx, :].astype(ml_dtypes.bfloat16)
    sinb = sin_p[d_idx, :].astype(ml_dtypes.bfloat16)

    rotm = np.zeros((128, 128), dtype=np.float32)
    for i in range(64):
        rotm[2 * i + 1, 2 * i] = -1.0   # rot[2i] = -in[2i+1]
        rotm[2 * i, 2 * i + 1] = 1.0    # rot[2i+1] = in[2i]
    rotm = rotm.astype(ml_dtypes.bfloat16)
    tri = np.tril(np.ones((128, 128), dtype=np.float32)).T  # [tk, tq] tk<=tq
    tri = tri.astype(ml_dtypes.bfloat16)

    def wtiles(W, sl):
        # lhsT tiles: [p, ko, e] with d = ko*128+p contracting
        Wt = np.ascontiguousarray(W[sl, :].T)        # [D, e]
        return np.ascontiguousarray(
            Wt.reshape(8, 128, Wt.shape[1]).transpose(1, 0, 2))

    WoT = np.ascontiguousarray(np.asarray(Wo, dtype=np.float32).T)
    wo_t = np.ascontiguousarray(WoT.reshape(8, 128, D).transpose(1, 0, 2))

    in_maps = []
    for c in range(N_CORES):
        sl = slice(EC * c, EC * (c + 1))
        in_maps.append({
            "xt": xt_bf,
            "wq": wtiles(np.asarray(Wq, np.float32), sl).astype(ml_dtypes.bfloat16),
            "wk": wtiles(np.asarray(Wk, np.float32), sl).astype(ml_dtypes.bfloat16),
            "wv": wtiles(np.asarray(Wv, np.float32), sl).astype(ml_dtypes.bfloat16),
            "wo": wo_t.astype(ml_dtypes.bfloat16),
            "cosb": cosb,
            "sinb": sinb,
            "rotm": rotm,
            "trimask": tri,
        })
    return in_maps


def kernel(x, Wq, Wk, Wv, Wo, token_positions):
    global last_results
    if "nc" not in _CACHE:
        _CACHE["nc"] = _build_program()
    nc = _CACHE["nc"]
    in_maps = _host_inputs(x, Wq, Wk, Wv, Wo, token_positions)
    res = bass_utils.run_bass_kernel_spmd(nc, in_maps, list(range(N_CORES)))
    last_results = res
    y = np.empty((B, T, D), dtype=np.float32)
    for c in range(N_CORES):
        yc = res.results[c]["y"]          # [B*256, D]
        for bb in range(B):
            for hf in range(2):
                r0 = 256 * bb + 128 * hf
                t0 = 1024 * hf + 128 * c
                y[bb, t0:t0 + 128, :] = yc[r0:r0 + 128, :]
    return y
